# revision 53
# baseline (speedup 1.0000x reference)
"""Causal self-attention (B=2, S=2048, D=1024, H=16) on 8 TRN2 NeuronCores.

Sharding: tensor-parallel over heads. Core c owns heads [2c, 2c+2) for both
batch elements: it projects Q/K/V for its 128 feature columns (rows of
Wq/Wk/Wv), runs causal attention for its 4 (b, h) pairs, and computes a
partial output projection against its 128 columns of Wo. The host sums the 8
partial outputs (the "all-reduce") and adds bo once.

Device layouts (per core):
  xT   (D, B*S)  bf16   x transposed, shared by all cores
  wqT/wkT/wvT (D, 128) bf16  (W[c*128:(c+1)*128, :].T)
  woT  (128, D)  bf16   (Wo[:, c*128:(c+1)*128].T)
  bq/bk (128, 1) f32, bvr (1, 128) bf16
Outputs (per core):
  attn  (4, S, S) bf16  p = b*2 + h_local; upper-triangular blocks stay 0
                        (ExternalOutput buffers are pre-zeroed)
  o_part (B*S, D) f32   partial output projection (no bias)
"""

import math

import numpy as np
import ml_dtypes

B, S, D, H = 2, 2048, 1024, 16
HD = D // H          # 64
NCORES = 8
HPC = H // NCORES    # heads per core = 2
FPC = HPC * HD       # feature cols per core = 128
T = B * S            # 4096 tokens

BF16 = ml_dtypes.bfloat16

_CACHE = {}
LAST_EXEC_TIME_NS = None
LAST_RESULTS = None


def _build_program(b, s, d, n_heads_local, scale, repeat=1,
                   sbufs=2, ptbufs=2, obufs=1, apbufs=3, projbufs=3,
                   attn_dma_engine="gpsimd", aot_engine="scalar",
                   osb_engine="vector", sc_exp=1024, avshare=True,
                   projbufs2=4, vppbufs=4, pnorm_engine="vector",
                   interleave_proj=True, ptb=8, vnat_engine="scalar",
                   o_bf16=True, qkbias_engine="scalar", aot_split=False,
                   iter_order="asc", prime_k=0, ablate=frozenset()):
    """Build the SPMD Bass program for one core. Shapes are parameters so a
    scaled-down version can be simulated cheaply. repeat>1 wraps the body in
    a For_i loop (same work each iteration) for slope-based HW timing."""
    import concourse.bass as bass
    import concourse.mybir as mybir
    import concourse.tile as tile
    from concourse import bacc
    from concourse.masks import make_identity, make_causal_mask

    f32 = mybir.dt.float32
    bf16 = mybir.dt.bfloat16

    t = b * s                      # tokens
    fpc = n_heads_local * 64       # local feature cols (partition dim), 128
    assert fpc == 128
    nkd = d // 128                 # contraction chunks for projections
    nqt = s // 128                 # q tiles per sequence
    SC = 512                       # matmul free-dim chunk (1 PSUM bank f32)

    # Bacc (not plain Bass): its compile() pass splits multi-wait
    # instructions via event semaphores — TRN2 allows 1 wait/instruction.
    nc = bacc.Bacc("TRN2")

    xT = nc.declare_dram_parameter("xT", [d, t], bf16, isOutput=False)
    wqT = nc.declare_dram_parameter("wqT", [d, fpc], bf16, isOutput=False)
    wkT = nc.declare_dram_parameter("wkT", [d, fpc], bf16, isOutput=False)
    wvT = nc.declare_dram_parameter("wvT", [d, fpc], bf16, isOutput=False)
    woT = nc.declare_dram_parameter("woT", [fpc, d], bf16, isOutput=False)
    bq = nc.declare_dram_parameter("bq", [fpc, 1], f32, isOutput=False)
    bk = nc.declare_dram_parameter("bk", [fpc, 1], f32, isOutput=False)
    bvr = nc.declare_dram_parameter("bvr", [1, fpc], bf16, isOutput=False)
    attn = nc.declare_dram_parameter("attn", [b * n_heads_local, s, s], bf16,
                                     isOutput=True)
    o_dt = bf16 if o_bf16 else f32
    o_part = nc.declare_dram_parameter("o_part", [t, d], o_dt, isOutput=True)

    attn_dma = {"gpsimd": nc.gpsimd, "sync": nc.sync,
                "scalar": nc.scalar}[attn_dma_engine]
    aot_eng = {"scalar": "scalar", "vector": "vector"}[aot_engine]
    osb_eng = {"scalar": "scalar", "vector": "vector"}[osb_engine]

    def copy_on(eng_name, out, in_):
        if eng_name == "scalar":
            nc.scalar.activation(out, in_,
                                 mybir.ActivationFunctionType.Copy)
        else:
            nc.vector.tensor_copy(out, in_)

    with tile.TileContext(nc) as tc:
        with tc.tile_pool(name="const", bufs=1) as const_pool:
            ident = const_pool.tile([128, 128], bf16)
            make_identity(nc, ident)
            cmask = const_pool.tile([128, 128], bf16)
            make_causal_mask(nc, cmask, mask_val=-1e9)
            ones1 = const_pool.tile([1, 128], bf16, tag="ones1")
            nc.vector.memset(ones1, 1.0)
            b_q = const_pool.tile([fpc, 1], f32, tag="bq")
            b_k = const_pool.tile([fpc, 1], f32, tag="bk")
            b_vr = const_pool.tile([1, fpc], bf16, tag="bvr")
            nc.sync.dma_start(out=b_q, in_=bq[:, :])
            nc.sync.dma_start(out=b_k, in_=bk[:, :])
            nc.sync.dma_start(out=b_vr, in_=bvr[:, :])
            wo_sb = const_pool.tile([fpc, d], bf16, tag="wo")
            nc.sync.dma_start(out=wo_sb, in_=woT[:, :])

            # Persistent activations: QT/KT (feat-part, token-free) and
            # V in natural layout (token-part, feat-free), all bf16.
            qt = const_pool.tile([fpc, t], bf16, tag="qt")
            kt = const_pool.tile([fpc, t], bf16, tag="kt")
            # v_nat[kp, jb, f]: token = jb*128 + kp (block jb counts b*nqt)
            v_nat = const_pool.tile([128, b * nqt, fpc], bf16, tag="vnat")

            def _phases():
                # attnpool + score-PSUM pool are opened OUTSIDE the proj
                # scope so the first prime_k softmax stages can be issued
                # inside phase 1 (their qt/kt chunks unlock progressively)
                # and fill PE/ACT gaps while x is still streaming in.
                with tc.tile_pool(name="attnpool", bufs=apbufs) as ap_pool:
                    _phase2_body(ap_pool)

            def _phase1(proj_pool, pp, vpp, softmax_stage, iters):
                    w_sb = {}
                    for name, drv in (("q", wqT), ("k", wkT), ("v", wvT)):
                        wtile = proj_pool.tile([128, nkd, fpc], bf16,
                                               tag=f"w{name}", name=f"w{name}")
                        nc.sync.dma_start(
                            out=wtile,
                            in_=drv.rearrange("(kc kp) f -> kp kc f", kp=128))
                        w_sb[name] = wtile
                    x_sb = proj_pool.tile([128, nkd, t], bf16, tag="xsb")
                    xT3 = xT.rearrange("(kc kp) t -> kp kc t", kp=128)
                    for kc in range(nkd):
                        # chunked so the first matmuls start after ~1/nkd
                        # of the load instead of the whole 8 MB
                        nc.sync.dma_start(out=x_sb[:, kc, :],
                                          in_=xT3[:, kc, :])
                    # QT / KT: feat on partitions, tokens free
                    def qk_proj(name, dst, bias, n):
                        ps = pp.tile([128, SC], f32, tag="proj",
                                     name="proj_ps")
                        for kc in range(nkd):
                            nc.tensor.matmul(
                                ps,
                                w_sb[name][:, kc, :],
                                x_sb[:, kc, bass.ts(n, SC)],
                                start=(kc == 0), stop=(kc == nkd - 1))
                        # bias add + cast to bf16 (per-partition bias AP)
                        if qkbias_engine == "vector":
                            nc.vector.tensor_scalar_add(
                                dst[:, bass.ts(n, SC)], ps, bias)
                        else:
                            nc.scalar.activation(
                                dst[:, bass.ts(n, SC)], ps,
                                mybir.ActivationFunctionType.Identity,
                                bias=bias)

                    # V directly in natural layout: tokens on partitions.
                    # v_tile = x_tile @ Wv.T + bv via PE (ones-row bias MM).
                    def v_proj(blk):
                        v_ps = vpp.tile([128, fpc], f32, tag="v",
                                        name="v_ps")
                        for kc in range(nkd):
                            nc.tensor.matmul(
                                v_ps,
                                x_sb[:, kc, blk * 128:(blk + 1) * 128],
                                w_sb["v"][:, kc, :],
                                start=(kc == 0), stop=False)
                        nc.tensor.matmul(
                            v_ps, ones1, b_vr, start=False, stop=True)
                        copy_on(vnat_engine, v_nat[:, blk, :], v_ps)

                    nvb = b * nqt // (t // SC)  # V blocks per n-chunk
                    primed = []
                    it_idx = 0
                    nchk = t // SC // b  # token chunks per batch
                    for n in range(t // SC):
                        qk_proj("q", qt, b_q, n)
                        qk_proj("k", kt, b_k, n)
                        for blk in range(n * nvb, (n + 1) * nvb):
                            v_proj(blk)
                        while it_idx < len(iters) and len(primed) < prime_k:
                            bb, i = iters[it_idx]
                            if bb * nchk + i // (SC // 128) > n:
                                break  # qt/kt chunks not issued yet
                            primed.append((bb, i, softmax_stage(bb, i)))
                            it_idx += 1
                    return primed, it_idx

            def _phase2_body(ap_pool):
                    sp = None  # assigned in the driver before first use

                    def softmax_stage(bb, i):
                        klen = (i + 1) * 128
                        nsc = math.ceil(klen / sc_exp)
                        p16s = []
                        for h in range(n_heads_local):
                            p_idx = bb * n_heads_local + h
                            q_lo = bb * s + i * 128
                            hsl = slice(h * 64, (h + 1) * 64)
                            exp16 = ap_pool.tile([128, s], bf16,
                                                 tag=f"exp{h}",
                                                 name=f"exp{h}")
                            p16 = ap_pool.tile([128, s], bf16,
                                               tag=f"p16{h}", name=f"p16{h}",
                                               bufs=max(apbufs, prime_k + 2))
                            sums = ap_pool.tile([128, 4], f32,
                                                tag=f"sums{h}",
                                                name=f"sums{h}")
                            rs = ap_pool.tile([128, 1], f32,
                                              tag=f"rs{h}", name=f"rs{h}")
                            for kc in range(nsc):
                                w = min(sc_exp, klen - kc * sc_exp)
                                s_ps = sp.tile([128, sc_exp], f32,
                                               tag="s", name="s_ps")
                                for sub in range(math.ceil(w / SC)):
                                    sw = min(SC, w - sub * SC)
                                    off = kc * sc_exp + sub * SC
                                    nc.tensor.matmul(
                                        s_ps[:, sub * SC:sub * SC + sw],
                                        qt[hsl, q_lo:q_lo + 128],
                                        kt[hsl, bb * s + off:
                                           bb * s + off + sw],
                                        start=True, stop=True)
                                if kc == nsc - 1:
                                    # mask the diagonal 128x128 block on PE:
                                    # accumulate I.T @ cmask (= cmask) onto
                                    # the scores
                                    nc.tensor.matmul(
                                        s_ps[:, w - 128:w], ident, cmask,
                                        start=False, stop=True,
                                        skip_group_check=True)
                                nc.scalar.activation(
                                    exp16[:, kc * sc_exp:kc * sc_exp + w],
                                    s_ps[:, 0:w],
                                    mybir.ActivationFunctionType.Exp,
                                    scale=scale,
                                    accum_out=sums[:, kc:kc + 1])
                            if nsc > 1:
                                nc.vector.reduce_sum(
                                    rs, sums[:, 0:nsc],
                                    axis=mybir.AxisListType.X)
                                nc.vector.reciprocal(rs, rs)
                            else:
                                nc.vector.reciprocal(rs, sums[:, 0:1])
                            if "pnorm" in ablate:
                                p16 = exp16
                            elif pnorm_engine == "gpsimd":
                                nc.gpsimd.tensor_scalar_mul(
                                    p16[:, 0:klen], exp16[:, 0:klen], rs)
                            else:
                                nc.vector.tensor_scalar_mul(
                                    p16[:, 0:klen], exp16[:, 0:klen], rs)
                            if "attndma" not in ablate:
                                attn_dma.dma_start(
                                    out=attn[p_idx, i * 128:(i + 1) * 128,
                                             0:klen],
                                    in_=p16[:, 0:klen])
                            p16s.append(p16)
                        return p16s

                    def av_stage(bb, i, p16s):
                        if "av" in ablate:
                            return
                        aot = ap_pool.tile([128, 128], bf16, tag="aot",
                                           name="aot")
                        if avshare:
                            # One PSUM bank for both heads: PE executes
                            # matmuls in issue order, so h0's accumulation
                            # group fully completes before h1's start=True
                            # clears the bank's has_written bits.
                            shared = avp.tile([128, 128], f32, tag="av",
                                              name="av")
                            avps = [shared] * n_heads_local
                        else:
                            avps = [avp.tile([128, 128], f32, tag=f"av{h}",
                                             name=f"av{h}")
                                    for h in range(n_heads_local)]
                        for h in range(n_heads_local):
                            hsl = slice(h * 64, (h + 1) * 64)
                            p16 = p16s[h]
                            for jb in range(math.ceil((i + 1) / ptb)):
                                nj = min(ptb, i + 1 - jb * ptb)
                                pt_ps = ptp.tile([128, ptb * 128], bf16,
                                                 tag="pt", name="pt_ps")
                                pt_sb = ap_pool.tile([128, ptb * 128], bf16,
                                                     tag=f"pt{h}",
                                                     name=f"pt{h}")
                                for jj in range(nj):
                                    j = jb * ptb + jj
                                    nc.tensor.transpose(
                                        pt_ps[:, jj * 128:(jj + 1) * 128],
                                        p16[:, j * 128:(j + 1) * 128],
                                        ident)
                                nc.vector.tensor_copy(
                                    pt_sb[:, 0:nj * 128],
                                    pt_ps[:, 0:nj * 128])
                                for jj in range(nj):
                                    j = jb * ptb + jj
                                    nc.tensor.matmul(
                                        avps[h][hsl, :],
                                        v_nat[:, bb * nqt + j, hsl],
                                        pt_sb[:, jj * 128:(jj + 1) * 128],
                                        start=(j == 0), stop=(j == i),
                                        tile_position=(0, h * 64))
                            eng = ("vector" if (aot_split and h == 1)
                                   else aot_eng)
                            copy_on(eng, aot[hsl, :], avps[h][hsl, :])
                        if "oproj" in ablate:
                            return
                        o_sb = ap_pool.tile([128, d], o_dt, tag="osb",
                                            name="o_sb")
                        for n in range(math.ceil(d / SC)):
                            w = min(SC, d - n * SC)
                            o_ps = op.tile([128, SC], f32, tag="o",
                                           name="o_ps")
                            nc.tensor.matmul(
                                o_ps[:, 0:w], aot,
                                wo_sb[:, n * SC:n * SC + w],
                                start=True, stop=True)
                            copy_on(osb_eng,
                                    o_sb[:, n * SC:n * SC + w], o_ps[:, 0:w])
                        nc.sync.dma_start(
                            out=o_part[bb * s + i * 128:
                                       bb * s + (i + 1) * 128, :],
                            in_=o_sb)

                    if iter_order == "desc":
                        # longest (largest i) first, alternating batches, so
                        # the pipeline drains on the cheapest iterations
                        iters = [(bb, i) for i in range(nqt - 1, -1, -1)
                                 for bb in range(b)]
                    else:
                        iters = [(bb, i) for bb in range(b)
                                 for i in range(nqt)]

                    # ---- Phase 1 (+ optionally primed softmax stages) ----
                    # With priming, the score-PSUM pool must coexist with the
                    # projection pools (4+2+2 banks); without it, projections
                    # get the deeper 4+4 chain config and sp opens later.
                    from contextlib import ExitStack
                    stack = ExitStack()
                    pj = min(projbufs2, 2) if prime_k else projbufs2
                    vb = min(vppbufs, 2) if prime_k else vppbufs
                    if prime_k:
                        sp = stack.enter_context(
                            tc.tile_pool(name="spsum", bufs=sbufs,
                                         space="PSUM"))
                    with tc.tile_pool(name="projpool", bufs=1) as proj_pool, \
                         tc.tile_pool(name="projpsum", bufs=pj,
                                      space="PSUM") as pp, \
                         tc.tile_pool(name="vpsum", bufs=vb,
                                      space="PSUM") as vpp:
                        primed, it_idx = _phase1(proj_pool, pp, vpp,
                                                 softmax_stage, iters)
                    if not prime_k:
                        sp = stack.enter_context(
                            tc.tile_pool(name="spsum", bufs=sbufs,
                                         space="PSUM"))

                    # ---- Phase 2: attention + output projection ----
                    # softmax_stage(n) is issued before av_stage(n-lag) so
                    # the in-order PE stream interleaves QK with
                    # transposes/AV/Oproj and never stalls on the ACT/DVE
                    # softmax chain latency.
                    from collections import deque
                    with tc.tile_pool(name="ptpsum", bufs=ptbufs,
                                      space="PSUM") as ptp, \
                         tc.tile_pool(name="avpsum", bufs=1,
                                      space="PSUM") as avp, \
                         tc.tile_pool(name="opsum", bufs=obufs,
                                      space="PSUM") as op:
                        pend = deque(primed)
                        lag = 1
                        while it_idx < len(iters):
                            bb, i = iters[it_idx]
                            it_idx += 1
                            pend.append((bb, i, softmax_stage(bb, i)))
                            if len(pend) > lag:
                                av_stage(*pend.popleft())
                        while pend:
                            av_stage(*pend.popleft())
                    stack.close()

            if repeat == 1:
                _phases()
            else:
                with tc.For_i(0, repeat, 1):
                    _phases()
    nc.finalize()
    return nc


def _get_program():
    key = (B, S, D, HPC)
    if key not in _CACHE:
        _CACHE[key] = _build_program(B, S, D, HPC, 1.0 / math.sqrt(HD))
    return _CACHE[key]


def kernel(x, Wq, bq, Wk, bk, Wv, bv, Wo, bo):
    from concourse.bass_utils import run_bass_kernel_spmd

    nc = _get_program()

    x = np.asarray(x, dtype=np.float32)
    xT = np.ascontiguousarray(x.reshape(T, D).T).astype(BF16)
    in_maps = []
    for c in range(NCORES):
        fsl = slice(c * FPC, (c + 1) * FPC)
        in_maps.append({
            "xT": xT,
            "wqT": np.ascontiguousarray(np.asarray(Wq)[fsl, :].T).astype(BF16),
            "wkT": np.ascontiguousarray(np.asarray(Wk)[fsl, :].T).astype(BF16),
            "wvT": np.ascontiguousarray(np.asarray(Wv)[fsl, :].T).astype(BF16),
            "woT": np.ascontiguousarray(np.asarray(Wo)[:, fsl].T).astype(BF16),
            "bq": np.asarray(bq)[fsl].reshape(FPC, 1).astype(np.float32),
            "bk": np.asarray(bk)[fsl].reshape(FPC, 1).astype(np.float32),
            "bvr": np.asarray(bv)[fsl].reshape(1, FPC).astype(BF16),
        })

    res = run_bass_kernel_spmd(nc, in_maps, core_ids=list(range(NCORES)))
    global LAST_EXEC_TIME_NS, LAST_RESULTS
    LAST_EXEC_TIME_NS = getattr(res, "exec_time_ns", None)
    LAST_RESULTS = res

    out = np.zeros((T, D), np.float32)
    attn_w = np.empty((B, H, S, S), np.float32)
    for c in range(NCORES):
        out += res.results[c]["o_part"]
        a = res.results[c]["attn"].astype(np.float32)
        attn_w[:, c * HPC:(c + 1) * HPC] = a.reshape(B, HPC, S, S)
    out += np.asarray(bo, dtype=np.float32)[None, :]
    return out.reshape(B, S, D), attn_w


# revision 58
# speedup vs baseline: 1.0148x; 1.0148x over previous
"""Causal self-attention (B=2, S=2048, D=1024, H=16) on 8 TRN2 NeuronCores.

Sharding: tensor-parallel over heads. Core c owns heads [2c, 2c+2) for both
batch elements: it projects Q/K/V for its 128 feature columns (rows of
Wq/Wk/Wv), runs causal attention for its 4 (b, h) pairs, and computes a
partial output projection against its 128 columns of Wo. The host sums the 8
partial outputs (the "all-reduce") and adds bo once.

Device layouts (per core):
  xT   (D, B*S)  bf16   x transposed, shared by all cores
  wqT/wkT/wvT (D, 128) bf16  (W[c*128:(c+1)*128, :].T)
  woT  (128, D)  bf16   (Wo[:, c*128:(c+1)*128].T)
  bq/bk (128, 1) f32, bvr (1, 128) bf16
Outputs (per core):
  attn  (4, S, S) bf16  p = b*2 + h_local; upper-triangular blocks stay 0
                        (ExternalOutput buffers are pre-zeroed)
  o_part (B*S, D) f32   partial output projection (no bias)
"""

import math

import numpy as np
import ml_dtypes

B, S, D, H = 2, 2048, 1024, 16
HD = D // H          # 64
NCORES = 8
HPC = H // NCORES    # heads per core = 2
FPC = HPC * HD       # feature cols per core = 128
T = B * S            # 4096 tokens

BF16 = ml_dtypes.bfloat16

_CACHE = {}
LAST_EXEC_TIME_NS = None
LAST_RESULTS = None


def _build_program(b, s, d, n_heads_local, scale, repeat=1,
                   sbufs=2, ptbufs=2, obufs=1, apbufs=3, projbufs=3,
                   attn_dma_engine="gpsimd", aot_engine="vector",
                   osb_engine="vector", sc_exp=1024, avshare=True,
                   projbufs2=4, vppbufs=4, pnorm_engine="vector",
                   interleave_proj=True, ptb=8, vnat_engine="scalar",
                   o_bf16=True, qkbias_engine="scalar", aot_split=False,
                   iter_order="asc", prime_k=0, osb_split=True,
                   ptcopy_split=False, ablate=frozenset()):
    """Build the SPMD Bass program for one core. Shapes are parameters so a
    scaled-down version can be simulated cheaply. repeat>1 wraps the body in
    a For_i loop (same work each iteration) for slope-based HW timing."""
    import concourse.bass as bass
    import concourse.mybir as mybir
    import concourse.tile as tile
    from concourse import bacc
    from concourse.masks import make_identity, make_causal_mask

    f32 = mybir.dt.float32
    bf16 = mybir.dt.bfloat16

    t = b * s                      # tokens
    fpc = n_heads_local * 64       # local feature cols (partition dim), 128
    assert fpc == 128
    nkd = d // 128                 # contraction chunks for projections
    nqt = s // 128                 # q tiles per sequence
    SC = 512                       # matmul free-dim chunk (1 PSUM bank f32)

    # Bacc (not plain Bass): its compile() pass splits multi-wait
    # instructions via event semaphores — TRN2 allows 1 wait/instruction.
    nc = bacc.Bacc("TRN2")

    xT = nc.declare_dram_parameter("xT", [d, t], bf16, isOutput=False)
    wqT = nc.declare_dram_parameter("wqT", [d, fpc], bf16, isOutput=False)
    wkT = nc.declare_dram_parameter("wkT", [d, fpc], bf16, isOutput=False)
    wvT = nc.declare_dram_parameter("wvT", [d, fpc], bf16, isOutput=False)
    woT = nc.declare_dram_parameter("woT", [fpc, d], bf16, isOutput=False)
    bq = nc.declare_dram_parameter("bq", [fpc, 1], f32, isOutput=False)
    bk = nc.declare_dram_parameter("bk", [fpc, 1], f32, isOutput=False)
    bvr = nc.declare_dram_parameter("bvr", [1, fpc], bf16, isOutput=False)
    attn = nc.declare_dram_parameter("attn", [b * n_heads_local, s, s], bf16,
                                     isOutput=True)
    o_dt = bf16 if o_bf16 else f32
    o_part = nc.declare_dram_parameter("o_part", [t, d], o_dt, isOutput=True)

    attn_dma = {"gpsimd": nc.gpsimd, "sync": nc.sync,
                "scalar": nc.scalar}[attn_dma_engine]
    aot_eng = {"scalar": "scalar", "vector": "vector"}[aot_engine]
    osb_eng = {"scalar": "scalar", "vector": "vector"}[osb_engine]

    def copy_on(eng_name, out, in_):
        if eng_name == "scalar":
            nc.scalar.activation(out, in_,
                                 mybir.ActivationFunctionType.Copy)
        else:
            nc.vector.tensor_copy(out, in_)

    with tile.TileContext(nc) as tc:
        with tc.tile_pool(name="const", bufs=1) as const_pool:
            ident = const_pool.tile([128, 128], bf16)
            make_identity(nc, ident)
            cmask = const_pool.tile([128, 128], bf16)
            make_causal_mask(nc, cmask, mask_val=-1e9)
            ones1 = const_pool.tile([1, 128], bf16, tag="ones1")
            nc.vector.memset(ones1, 1.0)
            b_q = const_pool.tile([fpc, 1], f32, tag="bq")
            b_k = const_pool.tile([fpc, 1], f32, tag="bk")
            b_vr = const_pool.tile([1, fpc], bf16, tag="bvr")
            nc.sync.dma_start(out=b_q, in_=bq[:, :])
            nc.sync.dma_start(out=b_k, in_=bk[:, :])
            nc.sync.dma_start(out=b_vr, in_=bvr[:, :])
            wo_sb = const_pool.tile([fpc, d], bf16, tag="wo")
            nc.sync.dma_start(out=wo_sb, in_=woT[:, :])

            # Persistent activations: QT/KT (feat-part, token-free) and
            # V in natural layout (token-part, feat-free), all bf16.
            qt = const_pool.tile([fpc, t], bf16, tag="qt")
            kt = const_pool.tile([fpc, t], bf16, tag="kt")
            # v_nat[kp, jb, f]: token = jb*128 + kp (block jb counts b*nqt)
            v_nat = const_pool.tile([128, b * nqt, fpc], bf16, tag="vnat")

            def _phases():
                # attnpool + score-PSUM pool are opened OUTSIDE the proj
                # scope so the first prime_k softmax stages can be issued
                # inside phase 1 (their qt/kt chunks unlock progressively)
                # and fill PE/ACT gaps while x is still streaming in.
                with tc.tile_pool(name="attnpool", bufs=apbufs) as ap_pool:
                    _phase2_body(ap_pool)

            def _phase1(proj_pool, pp, vpp, softmax_stage, iters):
                    w_sb = {}
                    for name, drv in (("q", wqT), ("k", wkT), ("v", wvT)):
                        wtile = proj_pool.tile([128, nkd, fpc], bf16,
                                               tag=f"w{name}", name=f"w{name}")
                        nc.sync.dma_start(
                            out=wtile,
                            in_=drv.rearrange("(kc kp) f -> kp kc f", kp=128))
                        w_sb[name] = wtile
                    x_sb = proj_pool.tile([128, nkd, t], bf16, tag="xsb")
                    xT3 = xT.rearrange("(kc kp) t -> kp kc t", kp=128)
                    for kc in range(nkd):
                        # chunked so the first matmuls start after ~1/nkd
                        # of the load instead of the whole 8 MB
                        nc.sync.dma_start(out=x_sb[:, kc, :],
                                          in_=xT3[:, kc, :])
                    # QT / KT: feat on partitions, tokens free
                    def qk_proj(name, dst, bias, n):
                        ps = pp.tile([128, SC], f32, tag="proj",
                                     name="proj_ps")
                        for kc in range(nkd):
                            nc.tensor.matmul(
                                ps,
                                w_sb[name][:, kc, :],
                                x_sb[:, kc, bass.ts(n, SC)],
                                start=(kc == 0), stop=(kc == nkd - 1))
                        # bias add + cast to bf16 (per-partition bias AP)
                        if qkbias_engine == "vector":
                            nc.vector.tensor_scalar_add(
                                dst[:, bass.ts(n, SC)], ps, bias)
                        else:
                            nc.scalar.activation(
                                dst[:, bass.ts(n, SC)], ps,
                                mybir.ActivationFunctionType.Identity,
                                bias=bias)

                    # V directly in natural layout: tokens on partitions.
                    # v_tile = x_tile @ Wv.T + bv via PE (ones-row bias MM).
                    def v_proj(blk):
                        v_ps = vpp.tile([128, fpc], f32, tag="v",
                                        name="v_ps")
                        for kc in range(nkd):
                            nc.tensor.matmul(
                                v_ps,
                                x_sb[:, kc, blk * 128:(blk + 1) * 128],
                                w_sb["v"][:, kc, :],
                                start=(kc == 0), stop=False)
                        nc.tensor.matmul(
                            v_ps, ones1, b_vr, start=False, stop=True)
                        copy_on(vnat_engine, v_nat[:, blk, :], v_ps)

                    nvb = b * nqt // (t // SC)  # V blocks per n-chunk
                    primed = []
                    it_idx = 0
                    nchk = t // SC // b  # token chunks per batch
                    for n in range(t // SC):
                        qk_proj("q", qt, b_q, n)
                        qk_proj("k", kt, b_k, n)
                        for blk in range(n * nvb, (n + 1) * nvb):
                            v_proj(blk)
                        while it_idx < len(iters) and len(primed) < prime_k:
                            bb, i = iters[it_idx]
                            if bb * nchk + i // (SC // 128) > n:
                                break  # qt/kt chunks not issued yet
                            primed.append((bb, i, softmax_stage(bb, i)))
                            it_idx += 1
                    return primed, it_idx

            def _phase2_body(ap_pool):
                    sp = None  # assigned in the driver before first use

                    def softmax_stage(bb, i):
                        klen = (i + 1) * 128
                        nsc = math.ceil(klen / sc_exp)
                        p16s = []
                        for h in range(n_heads_local):
                            p_idx = bb * n_heads_local + h
                            q_lo = bb * s + i * 128
                            hsl = slice(h * 64, (h + 1) * 64)
                            exp16 = ap_pool.tile([128, s], bf16,
                                                 tag=f"exp{h}",
                                                 name=f"exp{h}")
                            p16 = ap_pool.tile([128, s], bf16,
                                               tag=f"p16{h}", name=f"p16{h}",
                                               bufs=max(apbufs, prime_k + 2))
                            sums = ap_pool.tile([128, 4], f32,
                                                tag=f"sums{h}",
                                                name=f"sums{h}")
                            rs = ap_pool.tile([128, 1], f32,
                                              tag=f"rs{h}", name=f"rs{h}")
                            for kc in range(nsc):
                                w = min(sc_exp, klen - kc * sc_exp)
                                s_ps = sp.tile([128, sc_exp], f32,
                                               tag="s", name="s_ps")
                                for sub in range(math.ceil(w / SC)):
                                    sw = min(SC, w - sub * SC)
                                    off = kc * sc_exp + sub * SC
                                    nc.tensor.matmul(
                                        s_ps[:, sub * SC:sub * SC + sw],
                                        qt[hsl, q_lo:q_lo + 128],
                                        kt[hsl, bb * s + off:
                                           bb * s + off + sw],
                                        start=True, stop=True)
                                if kc == nsc - 1:
                                    # mask the diagonal 128x128 block on PE:
                                    # accumulate I.T @ cmask (= cmask) onto
                                    # the scores
                                    nc.tensor.matmul(
                                        s_ps[:, w - 128:w], ident, cmask,
                                        start=False, stop=True,
                                        skip_group_check=True)
                                nc.scalar.activation(
                                    exp16[:, kc * sc_exp:kc * sc_exp + w],
                                    s_ps[:, 0:w],
                                    mybir.ActivationFunctionType.Exp,
                                    scale=scale,
                                    accum_out=sums[:, kc:kc + 1])
                            if nsc > 1:
                                nc.vector.reduce_sum(
                                    rs, sums[:, 0:nsc],
                                    axis=mybir.AxisListType.X)
                                nc.vector.reciprocal(rs, rs)
                            else:
                                nc.vector.reciprocal(rs, sums[:, 0:1])
                            if "pnorm" in ablate:
                                p16 = exp16
                            elif pnorm_engine == "gpsimd":
                                nc.gpsimd.tensor_scalar_mul(
                                    p16[:, 0:klen], exp16[:, 0:klen], rs)
                            else:
                                nc.vector.tensor_scalar_mul(
                                    p16[:, 0:klen], exp16[:, 0:klen], rs)
                            if "attndma" not in ablate:
                                attn_dma.dma_start(
                                    out=attn[p_idx, i * 128:(i + 1) * 128,
                                             0:klen],
                                    in_=p16[:, 0:klen])
                            p16s.append(p16)
                        return p16s

                    def av_stage(bb, i, p16s):
                        if "av" in ablate:
                            return
                        aot = ap_pool.tile([128, 128], bf16, tag="aot",
                                           name="aot")
                        if avshare:
                            # One PSUM bank for both heads: PE executes
                            # matmuls in issue order, so h0's accumulation
                            # group fully completes before h1's start=True
                            # clears the bank's has_written bits.
                            shared = avp.tile([128, 128], f32, tag="av",
                                              name="av")
                            avps = [shared] * n_heads_local
                        else:
                            avps = [avp.tile([128, 128], f32, tag=f"av{h}",
                                             name=f"av{h}")
                                    for h in range(n_heads_local)]
                        for h in range(n_heads_local):
                            hsl = slice(h * 64, (h + 1) * 64)
                            p16 = p16s[h]
                            for jb in range(math.ceil((i + 1) / ptb)):
                                nj = min(ptb, i + 1 - jb * ptb)
                                pt_ps = ptp.tile([128, ptb * 128], bf16,
                                                 tag="pt", name="pt_ps")
                                pt_sb = ap_pool.tile([128, ptb * 128], bf16,
                                                     tag=f"pt{h}",
                                                     name=f"pt{h}")
                                for jj in range(nj):
                                    j = jb * ptb + jj
                                    nc.tensor.transpose(
                                        pt_ps[:, jj * 128:(jj + 1) * 128],
                                        p16[:, j * 128:(j + 1) * 128],
                                        ident)
                                if ptcopy_split and jb % 2:
                                    nc.scalar.activation(
                                        pt_sb[:, 0:nj * 128],
                                        pt_ps[:, 0:nj * 128],
                                        mybir.ActivationFunctionType.Copy)
                                else:
                                    nc.vector.tensor_copy(
                                        pt_sb[:, 0:nj * 128],
                                        pt_ps[:, 0:nj * 128])
                                for jj in range(nj):
                                    j = jb * ptb + jj
                                    nc.tensor.matmul(
                                        avps[h][hsl, :],
                                        v_nat[:, bb * nqt + j, hsl],
                                        pt_sb[:, jj * 128:(jj + 1) * 128],
                                        start=(j == 0), stop=(j == i),
                                        tile_position=(0, h * 64))
                            eng = ("vector" if (aot_split and h == 1)
                                   else aot_eng)
                            copy_on(eng, aot[hsl, :], avps[h][hsl, :])
                        if "oproj" in ablate:
                            return
                        o_sb = ap_pool.tile([128, d], o_dt, tag="osb",
                                            name="o_sb")
                        for n in range(math.ceil(d / SC)):
                            w = min(SC, d - n * SC)
                            o_ps = op.tile([128, SC], f32, tag="o",
                                           name="o_ps")
                            nc.tensor.matmul(
                                o_ps[:, 0:w], aot,
                                wo_sb[:, n * SC:n * SC + w],
                                start=True, stop=True)
                            oe = ("scalar" if (osb_split and n % 2)
                                  else osb_eng)
                            copy_on(oe,
                                    o_sb[:, n * SC:n * SC + w], o_ps[:, 0:w])
                        nc.sync.dma_start(
                            out=o_part[bb * s + i * 128:
                                       bb * s + (i + 1) * 128, :],
                            in_=o_sb)

                    if iter_order == "desc":
                        # longest (largest i) first, alternating batches, so
                        # the pipeline drains on the cheapest iterations
                        iters = [(bb, i) for i in range(nqt - 1, -1, -1)
                                 for bb in range(b)]
                    else:
                        iters = [(bb, i) for bb in range(b)
                                 for i in range(nqt)]

                    # ---- Phase 1 (+ optionally primed softmax stages) ----
                    # With priming, the score-PSUM pool must coexist with the
                    # projection pools (4+2+2 banks); without it, projections
                    # get the deeper 4+4 chain config and sp opens later.
                    from contextlib import ExitStack
                    stack = ExitStack()
                    pj = min(projbufs2, 2) if prime_k else projbufs2
                    vb = min(vppbufs, 2) if prime_k else vppbufs
                    if prime_k:
                        sp = stack.enter_context(
                            tc.tile_pool(name="spsum", bufs=sbufs,
                                         space="PSUM"))
                    with tc.tile_pool(name="projpool", bufs=1) as proj_pool, \
                         tc.tile_pool(name="projpsum", bufs=pj,
                                      space="PSUM") as pp, \
                         tc.tile_pool(name="vpsum", bufs=vb,
                                      space="PSUM") as vpp:
                        primed, it_idx = _phase1(proj_pool, pp, vpp,
                                                 softmax_stage, iters)
                    if not prime_k:
                        sp = stack.enter_context(
                            tc.tile_pool(name="spsum", bufs=sbufs,
                                         space="PSUM"))

                    # ---- Phase 2: attention + output projection ----
                    # softmax_stage(n) is issued before av_stage(n-lag) so
                    # the in-order PE stream interleaves QK with
                    # transposes/AV/Oproj and never stalls on the ACT/DVE
                    # softmax chain latency.
                    from collections import deque
                    with tc.tile_pool(name="ptpsum", bufs=ptbufs,
                                      space="PSUM") as ptp, \
                         tc.tile_pool(name="avpsum", bufs=1,
                                      space="PSUM") as avp, \
                         tc.tile_pool(name="opsum", bufs=obufs,
                                      space="PSUM") as op:
                        pend = deque(primed)
                        lag = 1
                        while it_idx < len(iters):
                            bb, i = iters[it_idx]
                            it_idx += 1
                            pend.append((bb, i, softmax_stage(bb, i)))
                            if len(pend) > lag:
                                av_stage(*pend.popleft())
                        while pend:
                            av_stage(*pend.popleft())
                    stack.close()

            if repeat == 1:
                _phases()
            else:
                with tc.For_i(0, repeat, 1):
                    _phases()
    nc.finalize()
    return nc


def _get_program():
    key = (B, S, D, HPC)
    if key not in _CACHE:
        _CACHE[key] = _build_program(B, S, D, HPC, 1.0 / math.sqrt(HD))
    return _CACHE[key]


def kernel(x, Wq, bq, Wk, bk, Wv, bv, Wo, bo):
    from concourse.bass_utils import run_bass_kernel_spmd

    nc = _get_program()

    x = np.asarray(x, dtype=np.float32)
    xT = np.ascontiguousarray(x.reshape(T, D).T).astype(BF16)
    in_maps = []
    for c in range(NCORES):
        fsl = slice(c * FPC, (c + 1) * FPC)
        in_maps.append({
            "xT": xT,
            "wqT": np.ascontiguousarray(np.asarray(Wq)[fsl, :].T).astype(BF16),
            "wkT": np.ascontiguousarray(np.asarray(Wk)[fsl, :].T).astype(BF16),
            "wvT": np.ascontiguousarray(np.asarray(Wv)[fsl, :].T).astype(BF16),
            "woT": np.ascontiguousarray(np.asarray(Wo)[:, fsl].T).astype(BF16),
            "bq": np.asarray(bq)[fsl].reshape(FPC, 1).astype(np.float32),
            "bk": np.asarray(bk)[fsl].reshape(FPC, 1).astype(np.float32),
            "bvr": np.asarray(bv)[fsl].reshape(1, FPC).astype(BF16),
        })

    res = run_bass_kernel_spmd(nc, in_maps, core_ids=list(range(NCORES)))
    global LAST_EXEC_TIME_NS, LAST_RESULTS
    LAST_EXEC_TIME_NS = getattr(res, "exec_time_ns", None)
    LAST_RESULTS = res

    out = np.zeros((T, D), np.float32)
    attn_w = np.empty((B, H, S, S), np.float32)
    for c in range(NCORES):
        out += res.results[c]["o_part"]
        a = res.results[c]["attn"].astype(np.float32)
        attn_w[:, c * HPC:(c + 1) * HPC] = a.reshape(B, HPC, S, S)
    out += np.asarray(bo, dtype=np.float32)[None, :]
    return out.reshape(B, S, D), attn_w
